# revision 12
# baseline (speedup 1.0000x reference)
"""Trainium2 Bass kernel for nn_DivEncLayer (per-slice Dense->ELU->LayerNorm->Dense).

Math (per batch b, slice q):
    z[b,q,u]  = sum_s x[b, q*S+s] * W1[q,s,u]            (+ b1 via ACT bias)
    h         = elu(z + b1)
    out[b,q]  = LN(h) . (gamma*W2) + beta.W2 + b2
              = (s3 - mu*A[q]) * rsqrt(var+eps) + C[q]
where (shift-invariance of LN lets us use hh = elu+1 = max(z+b1+1, min(exp(z+b1),1)) ):
    mu  = mean_u hh,  var = mean_u hh^2 - mu^2
    s3  = sum_u G[q,u]*hh[u],  G = gamma*W2,  A[q] = sum_u G,  C[q] = sum_u beta*W2 + b2

Device layout: partitions = (q-in-group, u) / (q,s); free dim = batch.
Host pre-transposes x to xT=[Q*S, B], shards batch across 8 cores.
All LayerNorm reductions run on the TensorEngine as block-diagonal matmuls.
"""

import sys
import numpy as np

for _p in ("/opt/trn_rl_repo", "/root/.axon_site/_ro/trn_rl_repo"):
    if _p not in sys.path:
        sys.path.insert(0, _p)

Q, S, U, B = 128, 8, 32, 16384
EPS = 1e-3
NCORES = 8
BL = B // NCORES          # 2048 batch rows per core
NBH = 2                   # batch halves per core
NB = BL // NBH            # 1024 batch per chunk
NW = NB // 512            # 512-wide matmul windows per chunk
NQG = Q // 4              # 32 groups of 4 slices
NSG = Q // 16             # 8 supergroups of 16 slices (xT DMA granularity)
GPB = 10                  # q-groups batched per stats PSUM tile (12 rows each)
NBATCH = (NQG + GPB - 1) // GPB

H_BF16 = False            # h / hsq / reduce weights in bf16 (2x DVE square)
E_BF16 = False            # exp output in bf16

_compiled = None


def _dt(bf16):
    import concourse.mybir as mybir
    return mybir.dt.bfloat16 if bf16 else mybir.dt.float32


def _build():
    from contextlib import ExitStack
    import concourse.bacc as bacc
    import concourse.mybir as mybir
    import concourse.tile as tile

    f32 = mybir.dt.float32
    h_dt = _dt(H_BF16)
    e_dt = _dt(E_BF16)
    Alu = mybir.AluOpType
    Act = mybir.ActivationFunctionType

    nc = bacc.Bacc()
    xT = nc.declare_dram_parameter("xT", [Q * S, BL], f32, isOutput=False)
    w1blk = nc.declare_dram_parameter("w1blk", [128, NSG * 128], f32, isOutput=False)
    redw = nc.declare_dram_parameter("redw", [128, NQG * 128], h_dt, isOutput=False)
    redwq = nc.declare_dram_parameter("redwq", [128, NQG * 128], h_dt, isOutput=False)
    b1c = nc.declare_dram_parameter("b1c", [128, NQG], f32, isOutput=False)
    b1p1c = nc.declare_dram_parameter("b1p1c", [128, NQG], f32, isOutput=False)
    acol = nc.declare_dram_parameter("acol", [128, 1], f32, isOutput=False)
    ccol = nc.declare_dram_parameter("ccol", [128, 1], f32, isOutput=False)
    out = nc.declare_dram_parameter("out", [Q, BL], f32, isOutput=True)

    with ExitStack() as ctx:
        tc = ctx.enter_context(tile.TileContext(nc))
        consts = ctx.enter_context(tc.tile_pool(name="consts", bufs=1))
        sb = ctx.enter_context(tc.tile_pool(name="sb", bufs=2))
        psz = ctx.enter_context(tc.tile_pool(name="psz", bufs=2, space="PSUM"))
        psst = ctx.enter_context(tc.tile_pool(name="psst", bufs=2, space="PSUM"))

        w1_sb = consts.tile([128, NSG * 128], f32)
        nc.sync.dma_start(w1_sb[:], w1blk[:])
        redw_sb = consts.tile([128, NQG * 128], h_dt)
        nc.sync.dma_start(redw_sb[:], redw[:])
        redwq_sb = consts.tile([128, NQG * 128], h_dt)
        nc.sync.dma_start(redwq_sb[:], redwq[:])
        b1_sb = consts.tile([128, NQG], f32)
        nc.sync.dma_start(b1_sb[:], b1c[:])
        b1p1_sb = consts.tile([128, NQG], f32)
        nc.sync.dma_start(b1p1_sb[:], b1p1c[:])
        a_sb = consts.tile([128, 1], f32)
        nc.sync.dma_start(a_sb[:], acol[:])
        c_sb = consts.tile([128, 1], f32)
        nc.sync.dma_start(c_sb[:], ccol[:])
        eps_sb = consts.tile([128, 1], f32)
        nc.vector.memset(eps_sb[:], EPS)

        # per-q stats accumulator: cols [mu | s3 | s2], partition = q
        s_sb = consts.tile([128, 3 * BL], f32)

        # Engine warm-ups: walrus codegen allows only ONE cross-engine
        # sync-wait on PE Matmult / ACT Activation instructions. Absorb
        # each const-DMA semaphore into each engine's vector clock with a
        # cheap op so real instructions never need two waits.
        warm_ps = psz.tile([128, 4], f32, tag="z")
        nc.tensor.matmul(
            warm_ps[:, 0:1], w1_sb[0:32, 0:128], w1_sb[0:32, 0:1],
            start=True, stop=True, tile_position=(0, 0),
        )
        nc.tensor.matmul(
            warm_ps[:, 1:2], redw_sb[:, 0:128], redw_sb[:, 0:1],
            start=True, stop=True,
        )
        nc.tensor.matmul(
            warm_ps[:, 2:3], redwq_sb[:, 0:128], redwq_sb[:, 0:1],
            start=True, stop=True,
        )
        wsb = consts.tile([128, 8], f32)
        nc.scalar.activation(wsb[:, 0:1], b1_sb[:, 0:1], Act.Exp)
        nc.vector.tensor_scalar_add(wsb[:, 1:2], b1p1_sb[:, 0:1], 0.0)
        nc.vector.tensor_scalar_add(wsb[:, 2:3], a_sb[:], 0.0)
        nc.vector.tensor_scalar_add(wsb[:, 3:4], c_sb[:], 0.0)

        for half in range(NBH):
            for sg in range(NSG):
                xt_sb = sb.tile([128, NB], f32, tag="xt")
                nc.sync.dma_start(
                    xt_sb[:], xT[128 * sg : 128 * (sg + 1), half * NB : (half + 1) * NB]
                )
                for gi in range(4):
                    g = 4 * sg + gi
                    k = g % GPB
                    ng = min(GPB, NQG - GPB * (g // GPB))  # groups in this batch
                    if k == 0:
                        st_ps = psst.tile([128, NB], f32, tag="st")

                    z_ps = psz.tile([128, NB], f32, tag="z")
                    for w in range(NW):
                        nc.tensor.matmul(
                            z_ps[:, 512 * w : 512 * (w + 1)],
                            w1_sb[32 * gi : 32 * (gi + 1), 128 * sg : 128 * (sg + 1)],
                            xt_sb[32 * gi : 32 * (gi + 1), 512 * w : 512 * (w + 1)],
                            start=True,
                            stop=True,
                            tile_position=(32 * gi, 0),
                        )

                    e_sb = sb.tile([128, NB], e_dt, tag="e")
                    nc.scalar.activation(
                        e_sb[:], z_ps[:], Act.Exp, bias=b1_sb[:, g : g + 1]
                    )
                    emin_sb = sb.tile([128, NB], e_dt, tag="emin")
                    nc.vector.tensor_scalar_min(emin_sb[:], e_sb[:], 1.0)
                    h_sb = sb.tile([128, NB], h_dt, tag="h")
                    nc.vector.scalar_tensor_tensor(
                        h_sb[:],
                        z_ps[:],
                        b1p1_sb[:, g : g + 1],
                        emin_sb[:],
                        Alu.add,
                        Alu.max,
                    )
                    hsq_sb = sb.tile([128, NB], h_dt, tag="hsq")
                    nc.vector.tensor_mul(hsq_sb[:], h_sb[:], h_sb[:])

                    for w in range(NW):
                        nc.tensor.matmul(
                            st_ps[:, 512 * w : 512 * (w + 1)],
                            redw_sb[:, 128 * g : 128 * (g + 1)],
                            h_sb[:, 512 * w : 512 * (w + 1)],
                            start=(k == 0),
                            stop=False,
                            skip_group_check=True,
                        )
                        nc.tensor.matmul(
                            st_ps[:, 512 * w : 512 * (w + 1)],
                            redwq_sb[:, 128 * g : 128 * (g + 1)],
                            hsq_sb[:, 512 * w : 512 * (w + 1)],
                            start=False,
                            stop=(k == ng - 1),
                            skip_group_check=True,
                        )

                    if k == ng - 1:
                        batch = g // GPB
                        stage = sb.tile([128, NB], f32, tag="stage")
                        nc.vector.tensor_copy(stage[:], st_ps[:])
                        # scatter stats rows (mu: 0..4ng, s3: 40.., s2: 80..)
                        # into per-q layout on s_sb; plain 2-D partition
                        # ranges (SBUF APs allow only one partition dim).
                        for st in range(3):
                            src = stage[40 * st : 40 * st + 4 * ng, :]
                            dst = s_sb[
                                40 * batch : 40 * batch + 4 * ng,
                                st * BL + half * NB : st * BL + (half + 1) * NB,
                            ]
                            nc.gpsimd.dma_start(dst, src)

        # ---- phase 2: per-q finalization ----
        # warm-up: absorb each stats-scatter DMA's semaphore one at a time
        # before any big DVE op reads s_sb (one wait per instruction max).
        w2sb = consts.tile([128, NBH * NBATCH * 3], f32)
        j = 0
        for st in range(3):
            for half in range(NBH):
                for batch in range(NBATCH):
                    # partition range grows from 0 so each read adds exactly
                    # one unobserved writer (engine APs must start at p=0/32/..)
                    np_ = 40 * batch + 1
                    nc.vector.tensor_scalar_add(
                        w2sb[0:np_, j : j + 1],
                        s_sb[
                            0:np_,
                            st * BL + half * NB : st * BL + half * NB + 1,
                        ],
                        0.0,
                    )
                    j += 1

        mu = s_sb[:, 0:BL]
        s3 = s_sb[:, BL : 2 * BL]
        s2 = s_sb[:, 2 * BL : 3 * BL]
        var = consts.tile([128, BL], f32)
        nc.vector.tensor_mul(var[:], mu[:], mu[:])
        nc.vector.tensor_sub(var[:], s2[:], var[:])
        sd = consts.tile([128, BL], f32)
        nc.scalar.activation(sd[:], var[:], Act.Sqrt, bias=eps_sb[:])
        nc.vector.reciprocal(out=sd[:], in_=sd[:])
        num = consts.tile([128, BL], f32)
        # num = mu*A - s3
        nc.vector.scalar_tensor_tensor(
            num[:], mu[:], a_sb[:], s3[:], Alu.mult, Alu.subtract
        )
        # o = (-num) * inv
        o_sb = consts.tile([128, BL], f32)
        nc.vector.scalar_tensor_tensor(
            o_sb[:], num[:], -1.0, sd[:], Alu.mult, Alu.mult
        )
        nc.vector.tensor_scalar_add(o_sb[:], o_sb[:], c_sb[:])
        nc.sync.dma_start(out[:], o_sb[:])

    nc.finalize()
    return nc


def _pack_consts(W1, b1, gamma, beta, W2, b2):
    f32 = np.float32
    W1 = np.asarray(W1, f32)
    b1 = np.asarray(b1, f32)
    gamma = np.asarray(gamma, f32)
    beta = np.asarray(beta, f32)
    W2 = np.asarray(W2, f32)
    b2 = np.asarray(b2, f32)

    G = (gamma * W2).astype(f32)               # [Q, U]
    A = G.sum(axis=1).astype(f32)              # [Q]
    C = ((beta * W2).sum(axis=1) + b2).astype(f32)

    w1blk = np.zeros((128, NSG * 128), f32)
    for sg in range(NSG):
        for gi in range(4):
            for qq in range(4):
                q = 16 * sg + 4 * gi + qq
                w1blk[
                    32 * gi + 8 * qq : 32 * gi + 8 * qq + 8,
                    128 * sg + 32 * qq : 128 * sg + 32 * qq + 32,
                ] = W1[q]

    np_h = np.dtype("bfloat16") if H_BF16 else f32
    try:
        import ml_dtypes  # noqa: F401
        np_h = np.dtype("bfloat16") if H_BF16 else f32
    except Exception:
        np_h = f32
    redw = np.zeros((128, NQG * 128), f32)
    redwq = np.zeros((128, NQG * 128), f32)
    for g in range(NQG):
        k = g % GPB
        for qq in range(4):
            q = 4 * g + qq
            rows = slice(32 * qq, 32 * qq + 32)
            j = 4 * k + qq
            redw[rows, 128 * g + j] = 1.0 / U          # mu rows 0..39
            redw[rows, 128 * g + 40 + j] = G[q]        # s3 rows 40..79
            redwq[rows, 128 * g + 80 + j] = 1.0 / U    # s2 rows 80..119

    b1c = np.zeros((128, NQG), f32)
    for g in range(NQG):
        for qq in range(4):
            b1c[32 * qq : 32 * qq + 32, g] = b1[4 * g + qq]
    b1p1c = (b1c + 1.0).astype(f32)

    acol = A.reshape(128, 1)
    ccol = C.reshape(128, 1)
    if H_BF16:
        import ml_dtypes
        redw = redw.astype(ml_dtypes.bfloat16)
        redwq = redwq.astype(ml_dtypes.bfloat16)
    return dict(
        w1blk=w1blk, redw=redw, redwq=redwq, b1c=b1c, b1p1c=b1p1c,
        acol=acol, ccol=ccol,
    )


def kernel(x, W1, b1, gamma, beta, W2, b2):
    global _compiled
    from concourse import bass_utils

    x = np.asarray(x, np.float32).reshape(B, Q * S)
    xT = np.ascontiguousarray(x.T)  # [Q*S, B]
    consts = _pack_consts(W1, b1, gamma, beta, W2, b2)

    if _compiled is None:
        _compiled = _build()
    nc = _compiled

    in_maps = []
    for c in range(NCORES):
        m = dict(consts)
        m["xT"] = np.ascontiguousarray(xT[:, c * BL : (c + 1) * BL])
        in_maps.append(m)

    res = bass_utils.run_bass_kernel_spmd(nc, in_maps, list(range(NCORES)))
    outs = [res.results[i]["out"] for i in range(NCORES)]  # each [Q, BL]
    full = np.concatenate(outs, axis=1)  # [Q, B]
    return np.ascontiguousarray(full.T).astype(np.float32)  # [B, Q]
